# revision 10
# baseline (speedup 1.0000x reference)
"""Multi-level (FPN) DeformRoIPool (zero-offset == aligned RoIAlign) for Trainium2.

Strategy (8 NeuronCores, SPMD, one Bass program):
- Host dedupes each ROI's bilinear footprint to its distinct feature pixels
  (K ~ 200-780 per ROI) and accumulates the per-(pixel, bin) weights into a
  dense [K, 49] matrix, so the device does no gathering at all: one plain
  contiguous HWDGE DMA per ROI slot brings [K_pad, 256] fp16 pixel rows into
  SBUF with K on partitions, and ceil(K/128) PE matmuls (lhsT = [128, 49]
  weights, rhs = [128, 256] pixels) accumulate the pooled [49, 256] result
  in PSUM. DVE casts PSUM to fp16 and a small DMA stores each slot.
- ROIs are sorted by K and dealt round-robin to the 8 cores so every core
  runs the same (static) chunk schedule with balanced work.
"""
import numpy as np

OUT = 7
SR = 2
STRIDES = (4, 8, 16, 32)
FINEST = 56.0
NLEV = 4
C = 256
N_ROIS = 256
N_CORES = 8
NSLOT = N_ROIS // N_CORES  # 32 roi slots per core
FEAT_SHAPES = [(2, 256, 200, 200), (2, 256, 100, 100), (2, 256, 50, 50), (2, 256, 25, 25)]


# ---------------------------------------------------------------------------
# BIR fix: this container's walrus rejects >1 embedded sem wait per
# instruction (2 on EventSemaphore). Split excess waits onto EventSemaphore
# carriers at serialization time.
# ---------------------------------------------------------------------------
def _install_bir_waitsplit():
    import orjson
    import concourse.bass as bass

    if getattr(bass.Bass, "_waitsplit_patched", False):
        return

    def _fix_blocks(blocks, counter):
        for blk in blocks:
            insts = blk.get("instructions")
            if insts:
                out = []
                for ins in insts:
                    si = ins.get("sync_info")
                    ow = (si or {}).get("on_wait") or []
                    limit = 2 if ins.get("opcode") == "EventSemaphore" else 1
                    if len(ow) > limit:
                        excess = ow[: len(ow) - limit]
                        si["on_wait"] = ow[len(ow) - limit:]
                        for i in range(0, len(excess), 2):
                            counter[0] += 1
                            out.append({
                                "name": f"I-waitsplit-{counter[0]}",
                                "opcode": "EventSemaphore",
                                "engine": ins["engine"],
                                "ins": [], "outs": [],
                                "debug": ins.get("debug", 0),
                                "sync_info": {"on_update": [], "on_wait": excess[i:i + 2]},
                            })
                    out.append(ins)
                blk["instructions"] = out
            if blk.get("blocks"):
                _fix_blocks(blk["blocks"], counter)

    orig = bass.Bass.to_json_bytes

    def to_json_bytes(self, *a, **kw):
        data = orig(self, *a, **kw)
        d = orjson.loads(data)
        counter = [0]
        for fn in d.get("functions", []):
            _fix_blocks(fn.get("blocks", []), counter)
        return orjson.dumps(d) if counter[0] else data

    bass.Bass.to_json_bytes = to_json_bytes
    bass.Bass._waitsplit_patched = True


# ---------------------------------------------------------------------------
# Host-side: per-ROI deduped pixel list + combined [K, 49] weights
# ---------------------------------------------------------------------------
def _roi_pixels(feats_T, rois):
    """Per ROI: (pix [K, C] fp16, wmat [K, 49] fp16) with K deduped pixels."""
    scale_wh = np.sqrt((rois[:, 3] - rois[:, 1]) * (rois[:, 4] - rois[:, 2]))
    with np.errstate(divide="ignore"):
        tl = np.clip(np.floor(np.log2(scale_wh / FINEST + 1e-6)), 0, NLEV - 1)
    tl = (tl + 1e-5).astype(np.int32)
    g = (np.arange(OUT, dtype=np.float64)[:, None]
         + (np.arange(SR, dtype=np.float64)[None, :] + 0.5) / SR)  # [OUT, SR]
    binmap = np.repeat(np.arange(OUT), SR)  # flat sample idx -> bin coordinate
    out = []
    for n in range(rois.shape[0]):
        l = int(tl[n])
        B, C_, H, W = FEAT_SHAPES[l]
        sc = 1.0 / STRIDES[l]
        x1 = rois[n, 1] * sc - 0.5
        y1 = rois[n, 2] * sc - 0.5
        rw = rois[n, 3] * sc - 0.5 - x1
        rh = rois[n, 4] * sc - 0.5 - y1
        y = (y1 + (rh / OUT) * g).reshape(-1)  # [14] sample y, idx iy=(i,si)
        x = (x1 + (rw / OUT) * g).reshape(-1)
        vy = (y > -1) & (y < H)
        vx = (x > -1) & (x < W)
        yc = np.clip(y, 0.0, H - 1)
        xc = np.clip(x, 0.0, W - 1)
        y0 = np.minimum(np.floor(yc).astype(np.int64), H - 1)
        x0 = np.minimum(np.floor(xc).astype(np.int64), W - 1)
        y1i = np.minimum(y0 + 1, H - 1)
        x1i = np.minimum(x0 + 1, W - 1)
        ly = yc - y0
        lx = xc - x0
        # corner coords/weights along each axis: [2, 14]
        cy = np.stack([y0, y1i])                      # [2, 14]
        wy = np.stack([1.0 - ly, ly])                 # [2, 14]
        cx = np.stack([x0, x1i])
        wx = np.stack([1.0 - lx, lx])
        valid = (vy[:, None] & vx[None, :]).astype(np.float64)  # [14, 14]
        # full contribution tensor [2, 14, 2, 14]
        w4 = (wy[:, :, None, None] * wx[None, None, :, :]) * valid[None, :, None, :] / (SR * SR)
        pid4 = cy[:, :, None, None] * W + cx[None, None, :, :]
        bins4 = np.broadcast_to(
            (binmap[:, None] * OUT + binmap[None, :])[None, :, None, :], w4.shape)
        pids = pid4.reshape(-1)
        ws = w4.reshape(-1)
        bs = bins4.reshape(-1)
        uniq, inv = np.unique(pids, return_inverse=True)
        K = len(uniq)
        wmat = np.zeros((K, OUT * OUT), np.float64)
        np.add.at(wmat, (inv, bs), ws)
        keep = wmat.any(axis=1)
        uniq, wmat = uniq[keep], wmat[keep]
        if len(uniq) == 0:  # fully-invalid roi -> zero output
            uniq = np.zeros(1, np.int64)
            wmat = np.zeros((1, OUT * OUT), np.float64)
        fT = feats_T[l][int(rois[n, 0])]  # [H, W, C]
        pix = fT.reshape(-1, C)[uniq].astype(np.float16)
        out.append((pix, wmat.astype(np.float16)))
    return out


CW = 49 + C  # combined per-chunk row: [49 weights | 256 pixels]


def _pack_cores(per_roi):
    """Sort ROIs by K desc, deal to 8 cores; return per-core combined
    [128, total_ch*CW] host arrays + shared chunk schedule + roi order."""
    ks = np.array([p.shape[0] for p, _ in per_roi])
    order = np.argsort(-ks, kind="stable")
    nch = []
    for s in range(NSLOT):
        kmax = max(per_roi[order[s * N_CORES + k]][0].shape[0] for k in range(N_CORES))
        nch.append(max(1, -(-int(kmax) // 128)))
    total_ch = sum(nch)
    wins = []
    for core in range(N_CORES):
        win = np.zeros((128, total_ch * CW), np.float16)
        off = 0
        for s in range(NSLOT):
            pix, wmat = per_roi[order[s * N_CORES + core]]
            K = pix.shape[0]
            for c in range(nch[s]):
                lo, hi = c * 128, min((c + 1) * 128, K)
                if lo >= K:
                    break
                win[0:hi - lo, (off + c) * CW:(off + c) * CW + 49] = wmat[lo:hi]
                win[0:hi - lo, (off + c) * CW + 49:(off + c + 1) * CW] = pix[lo:hi]
            off += nch[s]
        wins.append(win)
    return wins, nch, total_ch, order


# ---------------------------------------------------------------------------
# Device program
# ---------------------------------------------------------------------------
GROUP_CH = 12   # target chunks per DMA group
FIRST_CH = 6    # smaller first group so matmuls start sooner
OUT_BATCH = 4   # slots per staged output DMA


def _plan_groups(nch):
    """Pack consecutive slots into DMA groups of ~GROUP_CH chunks."""
    groups, cur, cnt = [], [], 0
    for s in range(NSLOT):
        cur.append(s)
        cnt += nch[s]
        if cnt >= (FIRST_CH if not groups else GROUP_CH):
            groups.append(cur)
            cur, cnt = [], 0
    if cur:
        groups.append(cur)
    return groups


def _build_program(nch, total_ch):
    import concourse.bacc as bacc
    import concourse.mybir as mybir
    import concourse.tile as tile

    _install_bir_waitsplit()
    nc = bacc.Bacc("TRN2", debug=False, enable_asserts=False, num_devices=N_CORES)

    win_d = nc.dram_tensor("win", [128, total_ch * CW], mybir.dt.float16, kind="ExternalInput")
    out_d = nc.dram_tensor("out", [NSLOT, 49 * C], mybir.dt.float16, kind="ExternalOutput")

    groups = _plan_groups(nch)
    slot_off = np.concatenate([[0], np.cumsum(nch)]).astype(int)

    with tile.TileContext(nc) as tc:
        with (
            tc.tile_pool(name="wp", bufs=len(groups)) as wp,
            tc.tile_pool(name="sp", bufs=4) as sp,
            tc.tile_pool(name="pp", bufs=4, space="PSUM") as pp,
        ):
            st = ps = None
            for g, slots in enumerate(groups):
                ring = nc.sync if g % 2 == 0 else nc.scalar
                g_lo, g_n = slot_off[slots[0]], sum(nch[s] for s in slots)
                wn = wp.tile([128, g_n * CW], mybir.dt.float16, tag="wn")
                ring.dma_start(wn[:], win_d[:, g_lo * CW:(g_lo + g_n) * CW])
                for s in slots:
                    j = s % OUT_BATCH
                    if j == 0:
                        st = sp.tile([49, OUT_BATCH * C], mybir.dt.float16, tag="st")
                        ps = pp.tile([49, OUT_BATCH * C], mybir.dt.float32, tag="ps")
                    n = nch[s]
                    for c in range(n):
                        k = slot_off[s] - g_lo + c
                        nc.tensor.matmul(
                            out=ps[:, j * C:(j + 1) * C],
                            lhsT=wn[:, k * CW:k * CW + 49],
                            rhs=wn[:, k * CW + 49:(k + 1) * CW],
                            start=(c == 0),
                            stop=(c == n - 1),
                        )
                    if j == OUT_BATCH - 1:
                        b = s // OUT_BATCH
                        if b % 2 == 0:
                            nc.vector.tensor_copy(st[:], ps[:])
                        else:
                            nc.scalar.activation(
                                st[:], ps[:], mybir.ActivationFunctionType.Copy)
                        (nc.sync if b % 2 == 0 else nc.scalar).dma_start(
                            out_d[b * OUT_BATCH:(b + 1) * OUT_BATCH].rearrange(
                                "s (b c) -> b s c", c=C),
                            st[:].rearrange("b (s c) -> b s c", c=C),
                        )
    nc.compile()
    return nc


def kernel(feat0, feat1, feat2, feat3, rois):
    from concourse.bass_utils import run_bass_kernel_spmd

    feats = [np.asarray(f, np.float32) for f in (feat0, feat1, feat2, feat3)]
    rois = np.asarray(rois, np.float32)
    feats_T = [np.ascontiguousarray(f.transpose(0, 2, 3, 1)) for f in feats]
    per_roi = _roi_pixels(feats_T, rois)
    wins, nch, total_ch, order = _pack_cores(per_roi)

    in_maps = [{"win": wins[core]} for core in range(N_CORES)]
    nc = _build_program(nch, total_ch)
    res = run_bass_kernel_spmd(nc, in_maps, core_ids=list(range(N_CORES)), trace=False)

    out = np.zeros((N_ROIS, C, OUT, OUT), np.float32)
    for core in range(N_CORES):
        o = res.results[core]["out"].astype(np.float32).reshape(NSLOT, 49, C)
        for s in range(NSLOT):
            out[order[s * N_CORES + core]] = o[s].T.reshape(C, OUT, OUT)
    return out


# Testing hook: emulate the device math in numpy (same win/wts host data).
def emulate(feat0, feat1, feat2, feat3, rois):
    feats = [np.asarray(f, np.float32) for f in (feat0, feat1, feat2, feat3)]
    rois = np.asarray(rois, np.float32)
    feats_T = [np.ascontiguousarray(f.transpose(0, 2, 3, 1)) for f in feats]
    per_roi = _roi_pixels(feats_T, rois)
    wins, nch, total_ch, order = _pack_cores(per_roi)
    out = np.zeros((N_ROIS, C, OUT, OUT), np.float32)
    for core in range(N_CORES):
        off = 0
        for s in range(NSLOT):
            n = nch[s]
            acc = np.zeros((49, C), np.float32)
            for c in range(n):
                blk = wins[core][:, (off + c) * CW:(off + c + 1) * CW].astype(np.float32)
                acc += blk[:, :49].T @ blk[:, 49:]
            st = acc.astype(np.float16).astype(np.float32)
            out[order[s * N_CORES + core]] = st.T.reshape(C, OUT, OUT)
            off += n
    return out


# revision 11
# speedup vs baseline: 1.1128x; 1.1128x over previous
"""Multi-level (FPN) DeformRoIPool (zero-offset == aligned RoIAlign) for Trainium2.

Strategy (8 NeuronCores, SPMD, one Bass program):
- Host dedupes each ROI's bilinear footprint to its distinct feature pixels
  (K ~ 200-780 per ROI) and accumulates the per-(pixel, bin) weights into a
  dense [K, 49] matrix, so the device does no gathering at all: one plain
  contiguous HWDGE DMA per ROI slot brings [K_pad, 256] fp16 pixel rows into
  SBUF with K on partitions, and ceil(K/128) PE matmuls (lhsT = [128, 49]
  weights, rhs = [128, 256] pixels) accumulate the pooled [49, 256] result
  in PSUM. DVE casts PSUM to fp16 and a small DMA stores each slot.
- ROIs are sorted by K and dealt round-robin to the 8 cores so every core
  runs the same (static) chunk schedule with balanced work.
"""
import numpy as np

OUT = 7
SR = 2
STRIDES = (4, 8, 16, 32)
FINEST = 56.0
NLEV = 4
C = 256
N_ROIS = 256
N_CORES = 8
NSLOT = N_ROIS // N_CORES  # 32 roi slots per core
FEAT_SHAPES = [(2, 256, 200, 200), (2, 256, 100, 100), (2, 256, 50, 50), (2, 256, 25, 25)]


# ---------------------------------------------------------------------------
# BIR fix: this container's walrus rejects >1 embedded sem wait per
# instruction (2 on EventSemaphore). Split excess waits onto EventSemaphore
# carriers at serialization time.
# ---------------------------------------------------------------------------
def _install_bir_waitsplit():
    import orjson
    import concourse.bass as bass

    if getattr(bass.Bass, "_waitsplit_patched", False):
        return

    def _fix_blocks(blocks, counter):
        for blk in blocks:
            insts = blk.get("instructions")
            if insts:
                out = []
                for ins in insts:
                    si = ins.get("sync_info")
                    ow = (si or {}).get("on_wait") or []
                    limit = 2 if ins.get("opcode") == "EventSemaphore" else 1
                    if len(ow) > limit:
                        excess = ow[: len(ow) - limit]
                        si["on_wait"] = ow[len(ow) - limit:]
                        for i in range(0, len(excess), 2):
                            counter[0] += 1
                            out.append({
                                "name": f"I-waitsplit-{counter[0]}",
                                "opcode": "EventSemaphore",
                                "engine": ins["engine"],
                                "ins": [], "outs": [],
                                "debug": ins.get("debug", 0),
                                "sync_info": {"on_update": [], "on_wait": excess[i:i + 2]},
                            })
                    out.append(ins)
                blk["instructions"] = out
            if blk.get("blocks"):
                _fix_blocks(blk["blocks"], counter)

    orig = bass.Bass.to_json_bytes

    def to_json_bytes(self, *a, **kw):
        data = orig(self, *a, **kw)
        d = orjson.loads(data)
        counter = [0]
        for fn in d.get("functions", []):
            _fix_blocks(fn.get("blocks", []), counter)
        return orjson.dumps(d) if counter[0] else data

    bass.Bass.to_json_bytes = to_json_bytes
    bass.Bass._waitsplit_patched = True


# ---------------------------------------------------------------------------
# Host-side: per-ROI deduped pixel list + combined [K, 49] weights
# ---------------------------------------------------------------------------
def _roi_pixels(feats_T, rois):
    """Per ROI: (pix [K, C] fp16, wmat [K, 49] fp16) with K deduped pixels."""
    scale_wh = np.sqrt((rois[:, 3] - rois[:, 1]) * (rois[:, 4] - rois[:, 2]))
    with np.errstate(divide="ignore"):
        tl = np.clip(np.floor(np.log2(scale_wh / FINEST + 1e-6)), 0, NLEV - 1)
    tl = (tl + 1e-5).astype(np.int32)
    g = (np.arange(OUT, dtype=np.float64)[:, None]
         + (np.arange(SR, dtype=np.float64)[None, :] + 0.5) / SR)  # [OUT, SR]
    binmap = np.repeat(np.arange(OUT), SR)  # flat sample idx -> bin coordinate
    out = []
    for n in range(rois.shape[0]):
        l = int(tl[n])
        B, C_, H, W = FEAT_SHAPES[l]
        sc = 1.0 / STRIDES[l]
        x1 = rois[n, 1] * sc - 0.5
        y1 = rois[n, 2] * sc - 0.5
        rw = rois[n, 3] * sc - 0.5 - x1
        rh = rois[n, 4] * sc - 0.5 - y1
        y = (y1 + (rh / OUT) * g).reshape(-1)  # [14] sample y, idx iy=(i,si)
        x = (x1 + (rw / OUT) * g).reshape(-1)
        vy = (y > -1) & (y < H)
        vx = (x > -1) & (x < W)
        yc = np.clip(y, 0.0, H - 1)
        xc = np.clip(x, 0.0, W - 1)
        y0 = np.minimum(np.floor(yc).astype(np.int64), H - 1)
        x0 = np.minimum(np.floor(xc).astype(np.int64), W - 1)
        y1i = np.minimum(y0 + 1, H - 1)
        x1i = np.minimum(x0 + 1, W - 1)
        ly = yc - y0
        lx = xc - x0
        # corner coords/weights along each axis: [2, 14]
        cy = np.stack([y0, y1i])                      # [2, 14]
        wy = np.stack([1.0 - ly, ly])                 # [2, 14]
        cx = np.stack([x0, x1i])
        wx = np.stack([1.0 - lx, lx])
        valid = (vy[:, None] & vx[None, :]).astype(np.float64)  # [14, 14]
        # full contribution tensor [2, 14, 2, 14]
        w4 = (wy[:, :, None, None] * wx[None, None, :, :]) * valid[None, :, None, :] / (SR * SR)
        pid4 = cy[:, :, None, None] * W + cx[None, None, :, :]
        bins4 = np.broadcast_to(
            (binmap[:, None] * OUT + binmap[None, :])[None, :, None, :], w4.shape)
        pids = pid4.reshape(-1)
        ws = w4.reshape(-1)
        bs = bins4.reshape(-1)
        uniq, inv = np.unique(pids, return_inverse=True)
        K = len(uniq)
        wmat = np.zeros((K, OUT * OUT), np.float64)
        np.add.at(wmat, (inv, bs), ws)
        keep = wmat.any(axis=1)
        uniq, wmat = uniq[keep], wmat[keep]
        if len(uniq) == 0:  # fully-invalid roi -> zero output
            uniq = np.zeros(1, np.int64)
            wmat = np.zeros((1, OUT * OUT), np.float64)
        fT = feats_T[l][int(rois[n, 0])]  # [H, W, C]
        pix = fT.reshape(-1, C)[uniq].astype(np.float16)
        out.append((pix, wmat.astype(np.float16)))
    return out


CW = 49 + C  # combined per-chunk row: [49 weights | 256 pixels]


def _pack_cores(per_roi):
    """Sort ROIs by K desc, deal to 8 cores; return per-core combined
    [128, total_ch*CW] host arrays + shared chunk schedule + roi order."""
    ks = np.array([p.shape[0] for p, _ in per_roi])
    order = np.argsort(-ks, kind="stable")
    nch = []
    for s in range(NSLOT):
        kmax = max(per_roi[order[s * N_CORES + k]][0].shape[0] for k in range(N_CORES))
        nch.append(max(1, -(-int(kmax) // 128)))
    total_ch = sum(nch)
    wins = []
    for core in range(N_CORES):
        win = np.zeros((128, total_ch * CW), np.float16)
        off = 0
        for s in range(NSLOT):
            pix, wmat = per_roi[order[s * N_CORES + core]]
            K = pix.shape[0]
            for c in range(nch[s]):
                lo, hi = c * 128, min((c + 1) * 128, K)
                if lo >= K:
                    break
                win[0:hi - lo, (off + c) * CW:(off + c) * CW + 49] = wmat[lo:hi]
                win[0:hi - lo, (off + c) * CW + 49:(off + c + 1) * CW] = pix[lo:hi]
            off += nch[s]
        wins.append(win)
    return wins, nch, total_ch, order


# ---------------------------------------------------------------------------
# Device program
# ---------------------------------------------------------------------------
GROUP_CH = 12   # target chunks per DMA group
FIRST_CH = 6    # smaller first group so matmuls start sooner
OUT_BATCH = 4   # slots per staged output DMA


def _plan_groups(nch):
    """Pack consecutive slots into DMA groups of ~GROUP_CH chunks."""
    groups, cur, cnt = [], [], 0
    for s in range(NSLOT):
        cur.append(s)
        cnt += nch[s]
        if cnt >= (FIRST_CH if not groups else GROUP_CH):
            groups.append(cur)
            cur, cnt = [], 0
    if cur:
        groups.append(cur)
    return groups


def _build_program(nch, total_ch):
    import concourse.bacc as bacc
    import concourse.mybir as mybir
    import concourse.tile as tile

    _install_bir_waitsplit()
    nc = bacc.Bacc("TRN2", debug=False, enable_asserts=False, num_devices=N_CORES)

    win_d = nc.dram_tensor("win", [128, total_ch * CW], mybir.dt.float16, kind="ExternalInput")
    out_d = nc.dram_tensor("out", [NSLOT, 49 * C], mybir.dt.float16, kind="ExternalOutput")

    groups = _plan_groups(nch)
    slot_off = np.concatenate([[0], np.cumsum(nch)]).astype(int)

    with tile.TileContext(nc) as tc:
        with (
            tc.tile_pool(name="wp", bufs=len(groups)) as wp,
            tc.tile_pool(name="sp", bufs=4) as sp,
            tc.tile_pool(name="pp", bufs=8, space="PSUM") as pp,
        ):
            st = None
            for g, slots in enumerate(groups):
                g_lo, g_n = slot_off[slots[0]], sum(nch[s] for s in slots)
                wn = wp.tile([128, g_n * CW], mybir.dt.float16, tag="wn")
                nc.sync.dma_start(wn[:], win_d[:, g_lo * CW:(g_lo + g_n) * CW])
                for s in slots:
                    j = s % OUT_BATCH
                    if j == 0:
                        st = sp.tile([49, OUT_BATCH * C], mybir.dt.float16, tag="st")
                    n = nch[s]
                    ps = pp.tile([49, C], mybir.dt.float32, tag="ps")
                    for c in range(n):
                        k = slot_off[s] - g_lo + c
                        nc.tensor.matmul(
                            out=ps[:, :],
                            lhsT=wn[:, k * CW:k * CW + 49],
                            rhs=wn[:, k * CW + 49:(k + 1) * CW],
                            start=(c == 0),
                            stop=(c == n - 1),
                        )
                    nc.vector.tensor_copy(st[:, j * C:(j + 1) * C], ps[:])
                    if j == OUT_BATCH - 1:
                        b = s // OUT_BATCH
                        nc.scalar.dma_start(
                            out_d[b * OUT_BATCH:(b + 1) * OUT_BATCH].rearrange(
                                "s (b c) -> b s c", c=C),
                            st[:].rearrange("b (s c) -> b s c", c=C),
                        )
    nc.compile()
    return nc


def kernel(feat0, feat1, feat2, feat3, rois):
    from concourse.bass_utils import run_bass_kernel_spmd

    feats = [np.asarray(f, np.float32) for f in (feat0, feat1, feat2, feat3)]
    rois = np.asarray(rois, np.float32)
    feats_T = [np.ascontiguousarray(f.transpose(0, 2, 3, 1)) for f in feats]
    per_roi = _roi_pixels(feats_T, rois)
    wins, nch, total_ch, order = _pack_cores(per_roi)

    in_maps = [{"win": wins[core]} for core in range(N_CORES)]
    nc = _build_program(nch, total_ch)
    res = run_bass_kernel_spmd(nc, in_maps, core_ids=list(range(N_CORES)), trace=False)

    out = np.zeros((N_ROIS, C, OUT, OUT), np.float32)
    for core in range(N_CORES):
        o = res.results[core]["out"].astype(np.float32).reshape(NSLOT, 49, C)
        for s in range(NSLOT):
            out[order[s * N_CORES + core]] = o[s].T.reshape(C, OUT, OUT)
    return out


# Testing hook: emulate the device math in numpy (same win/wts host data).
def emulate(feat0, feat1, feat2, feat3, rois):
    feats = [np.asarray(f, np.float32) for f in (feat0, feat1, feat2, feat3)]
    rois = np.asarray(rois, np.float32)
    feats_T = [np.ascontiguousarray(f.transpose(0, 2, 3, 1)) for f in feats]
    per_roi = _roi_pixels(feats_T, rois)
    wins, nch, total_ch, order = _pack_cores(per_roi)
    out = np.zeros((N_ROIS, C, OUT, OUT), np.float32)
    for core in range(N_CORES):
        off = 0
        for s in range(NSLOT):
            n = nch[s]
            acc = np.zeros((49, C), np.float32)
            for c in range(n):
                blk = wins[core][:, (off + c) * CW:(off + c + 1) * CW].astype(np.float32)
                acc += blk[:, :49].T @ blk[:, 49:]
            st = acc.astype(np.float16).astype(np.float32)
            out[order[s * N_CORES + core]] = st.T.reshape(C, OUT, OUT)
            off += n
    return out


# revision 14
# speedup vs baseline: 1.1313x; 1.0166x over previous
"""Multi-level (FPN) DeformRoIPool (zero-offset == aligned RoIAlign) for Trainium2.

Strategy (8 NeuronCores, SPMD, one Bass program):
- Host dedupes each ROI's bilinear footprint to its distinct feature pixels
  (K ~ 200-780 per ROI) and accumulates the per-(pixel, bin) weights into a
  dense [K, 49] matrix, so the device does no gathering at all: one plain
  contiguous HWDGE DMA per ROI slot brings [K_pad, 256] fp16 pixel rows into
  SBUF with K on partitions, and ceil(K/128) PE matmuls (lhsT = [128, 49]
  weights, rhs = [128, 256] pixels) accumulate the pooled [49, 256] result
  in PSUM. DVE casts PSUM to fp16 and a small DMA stores each slot.
- ROIs are sorted by K and dealt round-robin to the 8 cores so every core
  runs the same (static) chunk schedule with balanced work.
"""
import numpy as np

OUT = 7
SR = 2
STRIDES = (4, 8, 16, 32)
FINEST = 56.0
NLEV = 4
C = 256
N_ROIS = 256
N_CORES = 8
NSLOT = N_ROIS // N_CORES  # 32 roi slots per core
FEAT_SHAPES = [(2, 256, 200, 200), (2, 256, 100, 100), (2, 256, 50, 50), (2, 256, 25, 25)]


# ---------------------------------------------------------------------------
# BIR fix: this container's walrus rejects >1 embedded sem wait per
# instruction (2 on EventSemaphore). Split excess waits onto EventSemaphore
# carriers at serialization time.
# ---------------------------------------------------------------------------
def _install_bir_waitsplit():
    import orjson
    import concourse.bass as bass

    if getattr(bass.Bass, "_waitsplit_patched", False):
        return

    def _fix_blocks(blocks, counter):
        for blk in blocks:
            insts = blk.get("instructions")
            if insts:
                out = []
                for ins in insts:
                    si = ins.get("sync_info")
                    ow = (si or {}).get("on_wait") or []
                    limit = 2 if ins.get("opcode") == "EventSemaphore" else 1
                    if len(ow) > limit:
                        excess = ow[: len(ow) - limit]
                        si["on_wait"] = ow[len(ow) - limit:]
                        for i in range(0, len(excess), 2):
                            counter[0] += 1
                            out.append({
                                "name": f"I-waitsplit-{counter[0]}",
                                "opcode": "EventSemaphore",
                                "engine": ins["engine"],
                                "ins": [], "outs": [],
                                "debug": ins.get("debug", 0),
                                "sync_info": {"on_update": [], "on_wait": excess[i:i + 2]},
                            })
                    out.append(ins)
                blk["instructions"] = out
            if blk.get("blocks"):
                _fix_blocks(blk["blocks"], counter)

    orig = bass.Bass.to_json_bytes

    def to_json_bytes(self, *a, **kw):
        data = orig(self, *a, **kw)
        d = orjson.loads(data)
        counter = [0]
        for fn in d.get("functions", []):
            _fix_blocks(fn.get("blocks", []), counter)
        return orjson.dumps(d) if counter[0] else data

    bass.Bass.to_json_bytes = to_json_bytes
    bass.Bass._waitsplit_patched = True


# ---------------------------------------------------------------------------
# Host-side: per-ROI deduped pixel list + combined [K, 49] weights
# ---------------------------------------------------------------------------
def _roi_pixels(feats_T, rois):
    """Per ROI: (pix [K, C] fp16, wmat [K, 49] fp16) with K deduped pixels."""
    scale_wh = np.sqrt((rois[:, 3] - rois[:, 1]) * (rois[:, 4] - rois[:, 2]))
    with np.errstate(divide="ignore"):
        tl = np.clip(np.floor(np.log2(scale_wh / FINEST + 1e-6)), 0, NLEV - 1)
    tl = (tl + 1e-5).astype(np.int32)
    g = (np.arange(OUT, dtype=np.float64)[:, None]
         + (np.arange(SR, dtype=np.float64)[None, :] + 0.5) / SR)  # [OUT, SR]
    binmap = np.repeat(np.arange(OUT), SR)  # flat sample idx -> bin coordinate
    out = []
    for n in range(rois.shape[0]):
        l = int(tl[n])
        B, C_, H, W = FEAT_SHAPES[l]
        sc = 1.0 / STRIDES[l]
        x1 = rois[n, 1] * sc - 0.5
        y1 = rois[n, 2] * sc - 0.5
        rw = rois[n, 3] * sc - 0.5 - x1
        rh = rois[n, 4] * sc - 0.5 - y1
        y = (y1 + (rh / OUT) * g).reshape(-1)  # [14] sample y, idx iy=(i,si)
        x = (x1 + (rw / OUT) * g).reshape(-1)
        vy = (y > -1) & (y < H)
        vx = (x > -1) & (x < W)
        yc = np.clip(y, 0.0, H - 1)
        xc = np.clip(x, 0.0, W - 1)
        y0 = np.minimum(np.floor(yc).astype(np.int64), H - 1)
        x0 = np.minimum(np.floor(xc).astype(np.int64), W - 1)
        y1i = np.minimum(y0 + 1, H - 1)
        x1i = np.minimum(x0 + 1, W - 1)
        ly = yc - y0
        lx = xc - x0
        # corner coords/weights along each axis: [2, 14]
        cy = np.stack([y0, y1i])                      # [2, 14]
        wy = np.stack([1.0 - ly, ly])                 # [2, 14]
        cx = np.stack([x0, x1i])
        wx = np.stack([1.0 - lx, lx])
        valid = (vy[:, None] & vx[None, :]).astype(np.float64)  # [14, 14]
        # full contribution tensor [2, 14, 2, 14]
        w4 = (wy[:, :, None, None] * wx[None, None, :, :]) * valid[None, :, None, :] / (SR * SR)
        pid4 = cy[:, :, None, None] * W + cx[None, None, :, :]
        bins4 = np.broadcast_to(
            (binmap[:, None] * OUT + binmap[None, :])[None, :, None, :], w4.shape)
        pids = pid4.reshape(-1)
        ws = w4.reshape(-1)
        bs = bins4.reshape(-1)
        uniq, inv = np.unique(pids, return_inverse=True)
        K = len(uniq)
        wmat = np.zeros((K, OUT * OUT), np.float64)
        np.add.at(wmat, (inv, bs), ws)
        keep = wmat.any(axis=1)
        uniq, wmat = uniq[keep], wmat[keep]
        if len(uniq) == 0:  # fully-invalid roi -> zero output
            uniq = np.zeros(1, np.int64)
            wmat = np.zeros((1, OUT * OUT), np.float64)
        fT = feats_T[l][int(rois[n, 0])]  # [H, W, C]
        pix = fT.reshape(-1, C)[uniq].astype(np.float16)
        out.append((pix, wmat.astype(np.float16)))
    return out


CW = 49 + C  # combined per-chunk row: [49 weights | 256 pixels]


def _pack_cores(per_roi):
    """Sort ROIs by K desc, deal to 8 cores; return per-core combined
    [128, total_ch*CW] host arrays + shared chunk schedule + roi order."""
    ks = np.array([p.shape[0] for p, _ in per_roi])
    order = np.argsort(ks, kind="stable")  # small slots first: their per-slot
    # overheads hide under the DMA stream; the tail is one dense MM chain
    nch = []
    for s in range(NSLOT):
        kmax = max(per_roi[order[s * N_CORES + k]][0].shape[0] for k in range(N_CORES))
        nch.append(max(1, -(-int(kmax) // 128)))
    total_ch = sum(nch)
    wins = []
    for core in range(N_CORES):
        win = np.zeros((128, total_ch * CW), np.float16)
        off = 0
        for s in range(NSLOT):
            pix, wmat = per_roi[order[s * N_CORES + core]]
            K = pix.shape[0]
            for c in range(nch[s]):
                lo, hi = c * 128, min((c + 1) * 128, K)
                if lo >= K:
                    break
                win[0:hi - lo, (off + c) * CW:(off + c) * CW + 49] = wmat[lo:hi]
                win[0:hi - lo, (off + c) * CW + 49:(off + c + 1) * CW] = pix[lo:hi]
            off += nch[s]
        wins.append(win)
    return wins, nch, total_ch, order


# ---------------------------------------------------------------------------
# Device program
# ---------------------------------------------------------------------------
GROUP_CH = 12   # target chunks per DMA group
FIRST_CH = 6    # smaller first group so matmuls start sooner
OUT_BATCH = 4   # slots per staged output DMA


def _plan_groups(nch):
    """Pack consecutive slots into DMA groups of ~GROUP_CH chunks."""
    groups, cur, cnt = [], [], 0
    for s in range(NSLOT):
        cur.append(s)
        cnt += nch[s]
        if cnt >= (FIRST_CH if not groups else GROUP_CH):
            groups.append(cur)
            cur, cnt = [], 0
    if cur:
        groups.append(cur)
    return groups


def _build_program(nch, total_ch):
    import concourse.bacc as bacc
    import concourse.mybir as mybir
    import concourse.tile as tile

    _install_bir_waitsplit()
    nc = bacc.Bacc("TRN2", debug=False, enable_asserts=False, num_devices=N_CORES)

    win_d = nc.dram_tensor("win", [128, total_ch * CW], mybir.dt.float16, kind="ExternalInput")
    out_d = nc.dram_tensor("out", [NSLOT, 49 * C], mybir.dt.float16, kind="ExternalOutput")

    groups = _plan_groups(nch)
    slot_off = np.concatenate([[0], np.cumsum(nch)]).astype(int)

    with tile.TileContext(nc) as tc:
        with (
            tc.tile_pool(name="wp", bufs=len(groups)) as wp,
            tc.tile_pool(name="sp", bufs=4) as sp,
            tc.tile_pool(name="pp", bufs=8, space="PSUM") as pp,
        ):
            st = None
            for g, slots in enumerate(groups):
                g_lo, g_n = slot_off[slots[0]], sum(nch[s] for s in slots)
                wn = wp.tile([128, g_n * CW], mybir.dt.float16, tag="wn")
                nc.sync.dma_start(wn[:], win_d[:, g_lo * CW:(g_lo + g_n) * CW])
                for s in slots:
                    j = s % OUT_BATCH
                    if j == 0:
                        st = sp.tile([49, OUT_BATCH * C], mybir.dt.float16, tag="st")
                    n = nch[s]
                    ps = pp.tile([49, C], mybir.dt.float32, tag="ps")
                    for c in range(n):
                        k = slot_off[s] - g_lo + c
                        nc.tensor.matmul(
                            out=ps[:, :],
                            lhsT=wn[:, k * CW:k * CW + 49],
                            rhs=wn[:, k * CW + 49:(k + 1) * CW],
                            start=(c == 0),
                            stop=(c == n - 1),
                        )
                    nc.vector.tensor_copy(st[:, j * C:(j + 1) * C], ps[:])
                    if j == OUT_BATCH - 1:
                        b = s // OUT_BATCH
                        nc.scalar.dma_start(
                            out_d[b * OUT_BATCH:(b + 1) * OUT_BATCH].rearrange(
                                "s (b c) -> b s c", c=C),
                            st[:].rearrange("b (s c) -> b s c", c=C),
                        )
    nc.compile()
    return nc


def kernel(feat0, feat1, feat2, feat3, rois):
    from concourse.bass_utils import run_bass_kernel_spmd

    feats = [np.asarray(f, np.float32) for f in (feat0, feat1, feat2, feat3)]
    rois = np.asarray(rois, np.float32)
    feats_T = [np.ascontiguousarray(f.transpose(0, 2, 3, 1)) for f in feats]
    per_roi = _roi_pixels(feats_T, rois)
    wins, nch, total_ch, order = _pack_cores(per_roi)

    in_maps = [{"win": wins[core]} for core in range(N_CORES)]
    nc = _build_program(nch, total_ch)
    res = run_bass_kernel_spmd(nc, in_maps, core_ids=list(range(N_CORES)), trace=False)

    out = np.zeros((N_ROIS, C, OUT, OUT), np.float32)
    for core in range(N_CORES):
        o = res.results[core]["out"].astype(np.float32).reshape(NSLOT, 49, C)
        for s in range(NSLOT):
            out[order[s * N_CORES + core]] = o[s].T.reshape(C, OUT, OUT)
    return out


# Testing hook: emulate the device math in numpy (same win/wts host data).
def emulate(feat0, feat1, feat2, feat3, rois):
    feats = [np.asarray(f, np.float32) for f in (feat0, feat1, feat2, feat3)]
    rois = np.asarray(rois, np.float32)
    feats_T = [np.ascontiguousarray(f.transpose(0, 2, 3, 1)) for f in feats]
    per_roi = _roi_pixels(feats_T, rois)
    wins, nch, total_ch, order = _pack_cores(per_roi)
    out = np.zeros((N_ROIS, C, OUT, OUT), np.float32)
    for core in range(N_CORES):
        off = 0
        for s in range(NSLOT):
            n = nch[s]
            acc = np.zeros((49, C), np.float32)
            for c in range(n):
                blk = wins[core][:, (off + c) * CW:(off + c + 1) * CW].astype(np.float32)
                acc += blk[:, :49].T @ blk[:, 49:]
            st = acc.astype(np.float16).astype(np.float32)
            out[order[s * N_CORES + core]] = st.T.reshape(C, OUT, OUT)
            off += n
    return out


# revision 16
# speedup vs baseline: 1.1920x; 1.0537x over previous
"""Multi-level (FPN) DeformRoIPool (zero-offset == aligned RoIAlign) for Trainium2.

Strategy (8 NeuronCores, SPMD, one Bass program):
- Host dedupes each ROI's bilinear footprint to its distinct feature pixels
  (K ~ 200-780 per ROI) and accumulates the per-(pixel, bin) weights into a
  dense [K, 49] matrix, so the device does no gathering at all: one plain
  contiguous HWDGE DMA per ROI slot brings [K_pad, 256] fp16 pixel rows into
  SBUF with K on partitions, and ceil(K/128) PE matmuls (lhsT = [128, 49]
  weights, rhs = [128, 256] pixels) accumulate the pooled [49, 256] result
  in PSUM. DVE casts PSUM to fp16 and a small DMA stores each slot.
- ROIs are sorted by K and dealt round-robin to the 8 cores so every core
  runs the same (static) chunk schedule with balanced work.
"""
import numpy as np

OUT = 7
SR = 2
STRIDES = (4, 8, 16, 32)
FINEST = 56.0
NLEV = 4
C = 256
N_ROIS = 256
N_CORES = 8
NSLOT = N_ROIS // N_CORES  # 32 roi slots per core
FEAT_SHAPES = [(2, 256, 200, 200), (2, 256, 100, 100), (2, 256, 50, 50), (2, 256, 25, 25)]


# ---------------------------------------------------------------------------
# BIR fix: this container's walrus rejects >1 embedded sem wait per
# instruction (2 on EventSemaphore). Split excess waits onto EventSemaphore
# carriers at serialization time.
# ---------------------------------------------------------------------------
def _install_bir_waitsplit():
    import orjson
    import concourse.bass as bass

    if getattr(bass.Bass, "_waitsplit_patched", False):
        return

    def _fix_blocks(blocks, counter):
        for blk in blocks:
            insts = blk.get("instructions")
            if insts:
                out = []
                for ins in insts:
                    si = ins.get("sync_info")
                    ow = (si or {}).get("on_wait") or []
                    limit = 2 if ins.get("opcode") == "EventSemaphore" else 1
                    if len(ow) > limit:
                        excess = ow[: len(ow) - limit]
                        si["on_wait"] = ow[len(ow) - limit:]
                        for i in range(0, len(excess), 2):
                            counter[0] += 1
                            out.append({
                                "name": f"I-waitsplit-{counter[0]}",
                                "opcode": "EventSemaphore",
                                "engine": ins["engine"],
                                "ins": [], "outs": [],
                                "debug": ins.get("debug", 0),
                                "sync_info": {"on_update": [], "on_wait": excess[i:i + 2]},
                            })
                    out.append(ins)
                blk["instructions"] = out
            if blk.get("blocks"):
                _fix_blocks(blk["blocks"], counter)

    orig = bass.Bass.to_json_bytes

    def to_json_bytes(self, *a, **kw):
        data = orig(self, *a, **kw)
        d = orjson.loads(data)
        counter = [0]
        for fn in d.get("functions", []):
            _fix_blocks(fn.get("blocks", []), counter)
        return orjson.dumps(d) if counter[0] else data

    bass.Bass.to_json_bytes = to_json_bytes
    bass.Bass._waitsplit_patched = True


# ---------------------------------------------------------------------------
# Host-side: per-ROI deduped pixel list + combined [K, 49] weights
# ---------------------------------------------------------------------------
def _roi_pixels(feats_T, rois):
    """Per ROI: (pix [K, C] fp16, wmat [K, 49] fp16) with K deduped pixels."""
    scale_wh = np.sqrt((rois[:, 3] - rois[:, 1]) * (rois[:, 4] - rois[:, 2]))
    with np.errstate(divide="ignore"):
        tl = np.clip(np.floor(np.log2(scale_wh / FINEST + 1e-6)), 0, NLEV - 1)
    tl = (tl + 1e-5).astype(np.int32)
    g = (np.arange(OUT, dtype=np.float64)[:, None]
         + (np.arange(SR, dtype=np.float64)[None, :] + 0.5) / SR)  # [OUT, SR]
    binmap = np.repeat(np.arange(OUT), SR)  # flat sample idx -> bin coordinate
    out = []
    for n in range(rois.shape[0]):
        l = int(tl[n])
        B, C_, H, W = FEAT_SHAPES[l]
        sc = 1.0 / STRIDES[l]
        x1 = rois[n, 1] * sc - 0.5
        y1 = rois[n, 2] * sc - 0.5
        rw = rois[n, 3] * sc - 0.5 - x1
        rh = rois[n, 4] * sc - 0.5 - y1
        y = (y1 + (rh / OUT) * g).reshape(-1)  # [14] sample y, idx iy=(i,si)
        x = (x1 + (rw / OUT) * g).reshape(-1)
        vy = (y > -1) & (y < H)
        vx = (x > -1) & (x < W)
        yc = np.clip(y, 0.0, H - 1)
        xc = np.clip(x, 0.0, W - 1)
        y0 = np.minimum(np.floor(yc).astype(np.int64), H - 1)
        x0 = np.minimum(np.floor(xc).astype(np.int64), W - 1)
        y1i = np.minimum(y0 + 1, H - 1)
        x1i = np.minimum(x0 + 1, W - 1)
        ly = yc - y0
        lx = xc - x0
        # corner coords/weights along each axis: [2, 14]
        cy = np.stack([y0, y1i])                      # [2, 14]
        wy = np.stack([1.0 - ly, ly])                 # [2, 14]
        cx = np.stack([x0, x1i])
        wx = np.stack([1.0 - lx, lx])
        valid = (vy[:, None] & vx[None, :]).astype(np.float64)  # [14, 14]
        # full contribution tensor [2, 14, 2, 14]
        w4 = (wy[:, :, None, None] * wx[None, None, :, :]) * valid[None, :, None, :] / (SR * SR)
        pid4 = cy[:, :, None, None] * W + cx[None, None, :, :]
        bins4 = np.broadcast_to(
            (binmap[:, None] * OUT + binmap[None, :])[None, :, None, :], w4.shape)
        pids = pid4.reshape(-1)
        ws = w4.reshape(-1)
        bs = bins4.reshape(-1)
        uniq, inv = np.unique(pids, return_inverse=True)
        K = len(uniq)
        wmat = np.zeros((K, OUT * OUT), np.float64)
        np.add.at(wmat, (inv, bs), ws)
        keep = wmat.any(axis=1)
        uniq, wmat = uniq[keep], wmat[keep]
        if len(uniq) == 0:  # fully-invalid roi -> zero output
            uniq = np.zeros(1, np.int64)
            wmat = np.zeros((1, OUT * OUT), np.float64)
        fT = feats_T[l][int(rois[n, 0])]  # [H, W, C]
        pix = fT.reshape(-1, C)[uniq].astype(np.float16)
        out.append((pix, wmat.astype(np.float16)))
    return out


CW = 49 + C  # combined per-chunk row: [49 weights | 256 pixels]


def _pack_cores(per_roi):
    """Sort ROIs by K desc, deal to 8 cores; return per-core combined
    [128, total_ch*CW] host arrays + shared chunk schedule + roi order."""
    ks = np.array([p.shape[0] for p, _ in per_roi])
    order = np.argsort(ks, kind="stable")  # small slots first: their per-slot
    # overheads hide under the DMA stream; the tail is one dense MM chain
    nch = []
    for s in range(NSLOT):
        kmax = max(per_roi[order[s * N_CORES + k]][0].shape[0] for k in range(N_CORES))
        nch.append(max(1, -(-int(kmax) // 128)))
    total_ch = sum(nch)
    wins = []
    for core in range(N_CORES):
        win = np.zeros((128, total_ch * CW), np.float16)
        off = 0
        for s in range(NSLOT):
            pix, wmat = per_roi[order[s * N_CORES + core]]
            K = pix.shape[0]
            for c in range(nch[s]):
                lo, hi = c * 128, min((c + 1) * 128, K)
                if lo >= K:
                    break
                win[0:hi - lo, (off + c) * CW:(off + c) * CW + 49] = wmat[lo:hi]
                win[0:hi - lo, (off + c) * CW + 49:(off + c + 1) * CW] = pix[lo:hi]
            off += nch[s]
        wins.append(win)
    return wins, nch, total_ch, order


# ---------------------------------------------------------------------------
# Device program
# ---------------------------------------------------------------------------
GROUP_CH = 12   # target chunks per DMA group
FIRST_CH = 6    # smaller first group so matmuls start sooner
OUT_BATCH = 4   # slots per staged output DMA


def _plan_groups(nch):
    """Pack consecutive slots into DMA groups of ~GROUP_CH chunks."""
    groups, cur, cnt = [], [], 0
    for s in range(NSLOT):
        cur.append(s)
        cnt += nch[s]
        if cnt >= (FIRST_CH if not groups else GROUP_CH):
            groups.append(cur)
            cur, cnt = [], 0
    if cur:
        groups.append(cur)
    return groups


def _build_program(nch, total_ch):
    import concourse.bacc as bacc
    import concourse.mybir as mybir
    import concourse.tile as tile

    _install_bir_waitsplit()
    nc = bacc.Bacc("TRN2", debug=False, enable_asserts=False, num_devices=N_CORES)

    win_d = nc.dram_tensor("win", [128, total_ch * CW], mybir.dt.float16, kind="ExternalInput")
    out_d = nc.dram_tensor("out", [NSLOT, 49 * C], mybir.dt.float16, kind="ExternalOutput")

    groups = _plan_groups(nch)
    slot_off = np.concatenate([[0], np.cumsum(nch)]).astype(int)

    with tile.TileContext(nc) as tc:
        with (
            tc.tile_pool(name="wp", bufs=len(groups)) as wp,
            tc.tile_pool(name="sp", bufs=4) as sp,
            tc.tile_pool(name="pp", bufs=8, space="PSUM") as pp,
        ):
            st = None
            for g, slots in enumerate(groups):
                g_lo, g_n = slot_off[slots[0]], sum(nch[s] for s in slots)
                wn = wp.tile([128, g_n * CW], mybir.dt.float16, tag="wn")
                (nc.sync if g % 2 == 0 else nc.scalar).dma_start(
                    wn[:], win_d[:, g_lo * CW:(g_lo + g_n) * CW])
                for s in slots:
                    j = s % OUT_BATCH
                    if j == 0:
                        st = sp.tile([49, OUT_BATCH * C], mybir.dt.float16, tag="st")
                    n = nch[s]
                    ps = pp.tile([49, C], mybir.dt.float32, tag="ps")
                    for c in range(n):
                        k = slot_off[s] - g_lo + c
                        nc.tensor.matmul(
                            out=ps[:, :],
                            lhsT=wn[:, k * CW:k * CW + 49],
                            rhs=wn[:, k * CW + 49:(k + 1) * CW],
                            start=(c == 0),
                            stop=(c == n - 1),
                        )
                    nc.vector.tensor_copy(st[:, j * C:(j + 1) * C], ps[:])
                    if j == OUT_BATCH - 1:
                        b = s // OUT_BATCH
                        nc.gpsimd.dma_start(
                            out_d[b * OUT_BATCH:(b + 1) * OUT_BATCH].rearrange(
                                "s (b c) -> b s c", c=C),
                            st[:].rearrange("b (s c) -> b s c", c=C),
                        )
    nc.compile()
    return nc


def kernel(feat0, feat1, feat2, feat3, rois):
    from concourse.bass_utils import run_bass_kernel_spmd

    feats = [np.asarray(f, np.float32) for f in (feat0, feat1, feat2, feat3)]
    rois = np.asarray(rois, np.float32)
    feats_T = [np.ascontiguousarray(f.transpose(0, 2, 3, 1)) for f in feats]
    per_roi = _roi_pixels(feats_T, rois)
    wins, nch, total_ch, order = _pack_cores(per_roi)

    in_maps = [{"win": wins[core]} for core in range(N_CORES)]
    nc = _build_program(nch, total_ch)
    res = run_bass_kernel_spmd(nc, in_maps, core_ids=list(range(N_CORES)), trace=False)

    out = np.zeros((N_ROIS, C, OUT, OUT), np.float32)
    for core in range(N_CORES):
        o = res.results[core]["out"].astype(np.float32).reshape(NSLOT, 49, C)
        for s in range(NSLOT):
            out[order[s * N_CORES + core]] = o[s].T.reshape(C, OUT, OUT)
    return out


# Testing hook: emulate the device math in numpy (same win/wts host data).
def emulate(feat0, feat1, feat2, feat3, rois):
    feats = [np.asarray(f, np.float32) for f in (feat0, feat1, feat2, feat3)]
    rois = np.asarray(rois, np.float32)
    feats_T = [np.ascontiguousarray(f.transpose(0, 2, 3, 1)) for f in feats]
    per_roi = _roi_pixels(feats_T, rois)
    wins, nch, total_ch, order = _pack_cores(per_roi)
    out = np.zeros((N_ROIS, C, OUT, OUT), np.float32)
    for core in range(N_CORES):
        off = 0
        for s in range(NSLOT):
            n = nch[s]
            acc = np.zeros((49, C), np.float32)
            for c in range(n):
                blk = wins[core][:, (off + c) * CW:(off + c + 1) * CW].astype(np.float32)
                acc += blk[:, :49].T @ blk[:, 49:]
            st = acc.astype(np.float16).astype(np.float32)
            out[order[s * N_CORES + core]] = st.T.reshape(C, OUT, OUT)
            off += n
    return out
